# revision 25
# baseline (speedup 1.0000x reference)
"""Trainium2 Bass kernel for nn_DCGN_78967268704510.

Math: the reference's get_adjacent() builds a diagonal matrix (the faithful
buggy triple loop zeroes every off-diagonal), adds I, then symmetric-
normalizes; for a diagonal matrix D^-1/2 A D^-1/2 == I exactly (to fp32
ulps).  attn_pool feeds only get_adjacent, so the whole network collapses
to two fused stages:

  h   = leaky( (sum_p x[:,4s+p,:] * conv1_w[p,:]) @ prop1_W + prop1_B )
  out = leaky( (sum_p h[:,4t+p,:] * conv2_w[p,:]) @ prop2_W + prop2_B )

Sharding: pure data parallel, batch 64 -> 8 cores x 8 batches each.

v3 design notes (PE-throughput + DMA-overlap oriented):
  - single SP DMA ring, hand-interleaved: consts, x(0), x(1), w1p[0:6],
    x(2) quarters interleaved with w1p[6:16], x(3), stage-2 weights,
    x(4..7) -- every tensor arrives just before its consumer needs it.
  - x arrives as one 1MB DMA per (batch, f-quarter) in a [128, q*512+f]
    gathered layout; 8-deep quarter pool so buffer recycling is fine-
    grained and the tail batch's DMAs start early.
  - ALL elementwise work runs on DVE: any concurrent GpSimd tensor op
    degrades DVE from ~1.2us to ~3.4us per [128,1024] (measured), so
    GpSimd's 3.3us/op "help" is net-negative.  The conv1 scale is one
    DVE op per quarter using a stride-0 broadcast of the w1rep slice
    across the 4 node-groups.
  - b1rep/b2rep built by gpsimd.partition_broadcast at warmup time
    (bit-exact copy; DVE still idle then) from [1,*] rows: -0.8MB DMA.
  - ~28 warmup matmuls keep the PE busy from t~0 so the HAM clock gate
    opens early (cold PE runs at half clock).
  - software pipeline at quarter granularity; mm1(b-1) split into
    k0-7 / k8-15 halves emitted around the pool/transpose quarters so
    the in-order PE queue never waits on a late quarter.
  - all replicated weight tiles that feed matmuls stay host-built exact
    fp32 (replicating through fp32r matmuls costs ~5e-4 rounding and
    blows the 2e-2 gate).
"""
import sys

if '/opt/trn_rl_repo' not in sys.path:
    sys.path.insert(0, '/opt/trn_rl_repo')

import numpy as np

import concourse.bass as bass
import concourse.mybir as mybir
import concourse.tile as tile
from concourse.bass_utils import run_bass_kernel_spmd
from concourse.vector_clock import ScopedClock

N_CORES = 8
B, N, F, HID, NCLASS, P = 64, 512, 2048, 1100, 512, 4
BPC = B // N_CORES          # 8 batches per core
S = N // P                  # 128 stage-1 nodes
T = S // P                  # 32 stage-2 nodes
FT = F // 128               # 16 f-tiles
JT = (HID + 127) // 128     # 9 j-tiles, last has 76 rows
JLAST = HID - 128 * (JT - 1)
MM1_CHUNKS = (384, 384, 332)   # all >=256 so float32r runs at 1 cyc/row

FP32 = mybir.dt.float32
F32R = mybir.dt.float32r


class PatchedTileContext(tile.TileContext):
    """This container's walrus refuses ANY instruction carrying >1 sync
    wait (the TPB EVENTS struct has a single wait slot and the codegen
    won't split).  Split every multi-wait instruction into single-wait
    same-engine nops followed by the instruction with its last wait."""

    def _split_waits(self, inst):
        si = inst.sync_info
        waits = list(si.on_wait) if si and si.on_wait else []
        if len(waits) <= 1:
            return
        for w in waits[:-1]:
            nop = mybir.InstNoOp(
                name=self.nc.get_next_instruction_name(), ins=[], outs=[]
            )
            nop.engine = inst.engine
            nop.sync_info = mybir.SyncInfo(on_wait=[w], on_update=[])
            nop.bass_nofuse = True
            self._add_instruction(nop)
        inst.sync_info = mybir.SyncInfo(
            on_wait=[waits[-1]], on_update=list(si.on_update or [])
        )

    def _commit_instruction(self, inst, lazy_reg_writes=True):
        if inst.engine != mybir.EngineType.Unassigned:
            self._split_waits(inst)
        return super()._commit_instruction(inst, lazy_reg_writes)

    def _drain_and_barrier(self, tick_clock, wait_clock):
        probe = self.nc.sync.nop()
        wait_clock.add_sem_waits(
            probe.ins, ScopedClock({None: tick_clock.global_clock})
        )
        si = probe.ins.sync_info
        waits = list(si.on_wait) if si and si.on_wait else []
        if si and waits:
            probe.ins.sync_info = mybir.SyncInfo(
                on_wait=waits[:1], on_update=list(si.on_update or [])
            )
        for w in waits[1:]:
            n2 = self.nc.sync.nop()
            n2.ins.sync_info = mybir.SyncInfo(on_wait=[w], on_update=[])
        self.nc.sync.drain()
        self.nc.all_engine_barrier()
        assert self.sems is not None
        popped = self.nc._tile_sem_poison_stack.pop()
        assert popped is self._sem_poison
        self.nc.clear_and_free_semaphores(list(self.sems.allocated().values()))
        self.nc.all_engine_barrier()


def build_nc():
    nc = bass.Bass()
    xs_d = nc.dram_tensor('xs', [BPC, N, F], F32R, kind='ExternalInput')
    gq_d = nc.dram_tensor('gq', [128, 512], F32R, kind='ExternalInput')
    g2_d = nc.dram_tensor('g2', [128, 32], F32R, kind='ExternalInput')
    id_d = nc.dram_tensor('ident', [128, 128], F32R, kind='ExternalInput')
    w1rep_d = nc.dram_tensor('w1rep', [128, F], FP32, kind='ExternalInput')
    b1rep_d = nc.dram_tensor('b1rep', [128, HID], FP32, kind='ExternalInput')
    w2rep_d = nc.dram_tensor('w2rep', [128, HID], FP32, kind='ExternalInput')
    b2rep_d = nc.dram_tensor('b2rep', [128, NCLASS], FP32,
                             kind='ExternalInput')
    w1p_d = nc.dram_tensor('w1p', [F, HID], F32R, kind='ExternalInput')
    w2p_d = nc.dram_tensor('w2p', [HID, NCLASS], F32R, kind='ExternalInput')
    y_d = nc.dram_tensor('y', [BPC, T, NCLASS], FP32, kind='ExternalOutput')
    y_flat = y_d.rearrange('b t c -> (b t) c')   # [256, 512]

    with PatchedTileContext(nc) as tc:
        with (
            tc.tile_pool(name='wpool', bufs=1) as wpool,
            tc.tile_pool(name='cpool', bufs=8) as cpool,
            tc.tile_pool(name='xcpool', bufs=2) as xcpool,
            tc.tile_pool(name='xcTpool', bufs=6) as xcTpool,
            tc.tile_pool(name='h2pool', bufs=1) as h2pool,
            tc.tile_pool(name='hcTpool', bufs=1) as hcTpool,
            tc.tile_pool(name='opool', bufs=1) as opool,
            tc.tile_pool(name='pbig', bufs=4, space='PSUM') as pbigpool,
            tc.tile_pool(name='ph', bufs=3, space='PSUM') as phpool,
            tc.tile_pool(name='p2', bufs=1, space='PSUM') as p2pool,
        ):
            # ---- DMA order group 1: tiny consts + stage-1 scale/bias ----
            gq = wpool.tile([128, 512], F32R, tag='gq')
            nc.sync.dma_start(out=gq[:], in_=gq_d[:])
            g2 = wpool.tile([128, 32], F32R, tag='g2')
            nc.sync.dma_start(out=g2[:], in_=g2_d[:])
            ident = wpool.tile([128, 128], F32R, tag='ident')
            nc.sync.dma_start(out=ident[:], in_=id_d[:])
            w1rep = wpool.tile([128, F], FP32, tag='w1rep')
            nc.sync.dma_start(out=w1rep[:], in_=w1rep_d[:])
            b1rep = wpool.tile([128, HID], FP32, tag='b1rep')
            nc.sync.dma_start(out=b1rep[:], in_=b1rep_d[:])

            # ---- x loads: one 1MB DMA per (batch, f-quarter) ----
            cs = {}

            def load_cq(b, g):
                src = xs_d[b].rearrange('(q p) (g f) -> p g q f', p=128, g=4)
                c = cpool.tile([128, 2048], F32R, tag='c',
                               name=f'c_{b}_{g}')
                nc.sync.dma_start(
                    out=c.rearrange('p (q f) -> p q f', q=4),
                    in_=src[:, g],
                )
                cs[(b, g)] = c

            def load_c(b):
                for g in range(4):
                    load_cq(b, g)

            load_c(0)
            load_c(1)

            # ---- DMA order group 2: first chunk of w1p ----
            w1ps = []
            for k in range(FT):
                w1ps.append(wpool.tile([128, HID], F32R, tag=f'w1p{k}',
                                       name=f'w1p{k}'))

            def load_w1p(k0, k1):
                for k in range(k0, k1):
                    nc.sync.dma_start(
                        out=w1ps[k][:], in_=w1p_d[k * 128:(k + 1) * 128, :]
                    )

            load_w1p(0, 6)

            # stage-2 weights (loaded at iteration 1)
            w2rep = wpool.tile([128, HID], FP32, tag='w2rep')
            b2rep = wpool.tile([128, NCLASS], FP32, tag='b2rep')
            w2p = wpool.tile([128, JT * NCLASS], F32R, tag='w2p')

            def load_w2():
                nc.sync.dma_start(out=w2rep[:], in_=w2rep_d[:])
                nc.sync.dma_start(out=b2rep[:], in_=b2rep_d[:])
                for m in range(JT):
                    rows = 128 if m < JT - 1 else JLAST
                    nc.sync.dma_start(
                        out=w2p[0:rows, m * NCLASS:(m + 1) * NCLASS],
                        in_=w2p_d[m * 128:m * 128 + rows, :],
                    )

            # ---- PE warmup: dummy matmuls during the initial DMA wait so
            #      the HAM clock gate reaches K=8/8 before batch 0 ----
            for w in range(4):
                warm_ps = p2pool.tile([128, 512], FP32, tag='p2',
                                      name=f'warm{w}')
                for i in range(7):
                    nc.tensor.matmul(warm_ps[:], ident[:], gq[:],
                                     start=(i == 0), stop=(i == 6))

            # ---- per-batch stages (quarter granularity) ----

            def scale_q(b, g):
                cv = cs[(b, g)].rearrange('p (q f) -> p q f', q=4)
                wsl = w1rep[:, g * 512:(g + 1) * 512]
                wb = wsl.rearrange('p (q f) -> p q f', q=1)
                nc.vector.tensor_mul(cv, cv, wb.broadcast_to([128, 4, 512]))

            def scale_c(b):
                for g in range(4):
                    scale_q(b, g)

            xcs = {}
            xcTs = {}
            h2s = {}
            hcT = [None, None]

            def pool_q(b, g):
                """pool f-quarter g of batch b -> xc (SBUF [128,512])."""
                cv = cs[(b, g)].rearrange('p (q f) -> p q f', q=4)
                pp = pbigpool.tile([128, 512], FP32, tag='pbig',
                                   name=f'pp_{b}_{g}')
                for q in range(4):
                    nc.tensor.matmul(
                        pp[:],
                        gq[:, 128 * q:128 * (q + 1)],
                        cv[:, q],
                        start=(q == 0), stop=(q == 3),
                    )
                xc = xcpool.tile([128, 512], F32R, tag='xc',
                                 name=f'xc_{b}_{g}')
                nc.scalar.copy(out=xc[:], in_=pp[:])
                xcs[(b, g)] = xc

            def transpose_q(b, g):
                xc = xcs.pop((b, g))
                pt = pbigpool.tile([128, 512], F32R, tag='pbig',
                                   name=f'pt_{b}_{g}')
                for kk in range(4):
                    nc.tensor.transpose(
                        pt[:, 128 * kk:128 * (kk + 1)],
                        xc[:, kk * 128:(kk + 1) * 128],
                        ident[:],
                    )
                xcT = xcTpool.tile([128, 512], F32R, tag='xcT',
                                   name=f'xcT_{b}_{g}')
                nc.scalar.copy(out=xcT[:], in_=pt[:])
                xcTs[(b, g)] = xcT

            phs = {}

            def mm1_a(b):
                """mm1 k=0..7 for batch b (consumes xcT quarters 0-1)."""
                ph = []
                c0 = 0
                for cn in MM1_CHUNKS:
                    pht = phpool.tile([128, cn], FP32, tag='ph',
                                      name=f'ph_{b}_{c0}')
                    for k in range(8):
                        nc.tensor.matmul(
                            pht[:],
                            xcTs[(b, k // 4)][:, (k % 4) * 128:
                                              (k % 4 + 1) * 128],
                            w1ps[k][:, c0:c0 + cn],
                            start=(k == 0), stop=False,
                        )
                    ph.append((pht, c0, cn))
                    c0 += cn
                phs[b] = ph

            def mm1_b(b):
                """mm1 k=8..15 + bias + leaky + conv2-scale for batch b."""
                ph = phs.pop(b)
                h2 = h2pool.tile([128, HID], F32R, tag='h2', name=f'h2_{b}')
                for pht, c0, cn in ph:
                    for k in range(8, FT):
                        nc.tensor.matmul(
                            pht[:],
                            xcTs[(b, k // 4)][:, (k % 4) * 128:
                                              (k % 4 + 1) * 128],
                            w1ps[k][:, c0:c0 + cn],
                            start=False, stop=(k == FT - 1),
                        )
                    nc.vector.tensor_add(
                        h2[:, c0:c0 + cn], pht[:], b1rep[:, c0:c0 + cn]
                    )
                    nc.scalar.activation(
                        h2[:, c0:c0 + cn], h2[:, c0:c0 + cn],
                        mybir.ActivationFunctionType.Lrelu, alpha=0.01,
                    )
                for g in range(4):
                    xcTs.pop((b, g))
                nc.vector.tensor_mul(h2[:], h2[:], w2rep[:])
                h2s[b] = h2

            def stage2(b):
                """stage-2 pool-transpose + (every 4th) mm2 + store."""
                h2 = h2s.pop(b)
                pt2 = p2pool.tile([128, JT * T], FP32, tag='p2',
                                  name=f'pt2_{b}')
                for m in range(JT):
                    rows = 128 if m < JT - 1 else JLAST
                    nc.tensor.matmul(
                        pt2[0:rows, m * T:(m + 1) * T],
                        h2[:, m * 128:m * 128 + rows],
                        g2[:],
                        start=True, stop=True,
                    )
                g, bg = divmod(b, 4)
                if bg == 0:
                    hcT[g] = hcTpool.tile(
                        [128, JT * 128], F32R, tag='hcT', name=f'hcT{g}'
                    )
                dst = hcT[g].rearrange('p (m c) -> p m c', m=JT)[
                    :, :, 32 * bg:32 * (bg + 1)
                ]
                src = pt2[:].rearrange('p (m c) -> p m c', m=JT)
                # region-exact: rows [JLAST:128] of the last j-block are
                # never written by the pt2 matmuls
                nc.scalar.copy(out=dst[:, 0:JT - 1], in_=src[:, 0:JT - 1])
                nc.scalar.copy(out=dst[0:JLAST, JT - 1:JT],
                               in_=src[0:JLAST, JT - 1:JT])

                if bg == 3:
                    po = p2pool.tile([128, NCLASS], FP32, tag='p2',
                                     name=f'po_{g}')
                    for m in range(JT):
                        rows = 128 if m < JT - 1 else JLAST
                        nc.tensor.matmul(
                            po[:],
                            hcT[g][0:rows, m * 128:(m + 1) * 128],
                            w2p[0:rows, m * NCLASS:(m + 1) * NCLASS],
                            start=(m == 0), stop=(m == JT - 1),
                        )
                    ob = opool.tile([128, NCLASS], FP32, tag='ob',
                                    name=f'ob_{g}')
                    nc.vector.tensor_add(ob[:], po[:], b2rep[:])
                    nc.scalar.activation(
                        ob[:], ob[:],
                        mybir.ActivationFunctionType.Lrelu, alpha=0.01,
                    )
                    nc.sync.dma_start(
                        out=y_flat[128 * g:128 * (g + 1), :], in_=ob[:]
                    )

            # ---- software pipeline ----
            scale_c(0)
            scale_c(1)
            for b in range(BPC):
                if b == 0:
                    # interleave the rest of w1p with x(2) quarters so
                    # neither stream starves its consumer
                    load_cq(2, 0)
                    load_cq(2, 1)
                    load_w1p(6, 11)
                    load_cq(2, 2)
                    load_cq(2, 3)
                    load_w1p(11, FT)
                elif b + 2 < BPC:
                    load_c(b + 2)
                if b == 1:
                    load_w2()
                if b >= 1:
                    mm1_a(b - 1)
                pool_q(b, 0)
                transpose_q(b, 0)
                pool_q(b, 1)
                transpose_q(b, 1)
                if b >= 1:
                    mm1_b(b - 1)
                pool_q(b, 2)
                transpose_q(b, 2)
                pool_q(b, 3)
                transpose_q(b, 3)
                if b >= 1:
                    stage2(b - 1)
                if b + 2 < BPC:
                    scale_c(b + 2)
                for g in range(4):
                    cs.pop((b, g))
            mm1_a(BPC - 1)
            mm1_b(BPC - 1)
            stage2(BPC - 1)
    return nc


def _host_consts(conv1_w, pool1_w, pool1_b, prop1_W, prop1_B,
                 conv2_w, pool2_w, pool2_b, prop2_W, prop2_B):
    f32 = lambda a: np.ascontiguousarray(np.asarray(a, dtype=np.float32))
    gq = np.zeros((128, 512), dtype=np.float32)
    n = np.arange(128)
    for q in range(4):
        gq[n, 128 * q + 32 * q + n // 4] = 1.0
    g2 = np.zeros((128, 32), dtype=np.float32)
    g2[n, n // 4] = 1.0
    return {
        'gq': gq,
        'g2': g2,
        'ident': np.eye(128, dtype=np.float32),
        'w1rep': f32(np.tile(np.asarray(conv1_w), (32, 1))),
        'b1rep': f32(np.broadcast_to(np.asarray(prop1_B), (128, HID))),
        'w2rep': f32(np.tile(np.asarray(conv2_w), (32, 1))),
        'b2rep': f32(np.broadcast_to(np.asarray(prop2_B), (128, NCLASS))),
        'w1p': f32(prop1_W),
        'w2p': f32(prop2_W),
    }


_COMPILED = {}


def run_on_cores(inputs, trace=False, **run_kwargs):
    x = np.ascontiguousarray(np.asarray(inputs['x'], dtype=np.float32))
    consts = _host_consts(**{k: v for k, v in inputs.items()
                             if k not in ('x', 'pooling_size')})
    if 'nc' not in _COMPILED:
        _COMPILED['nc'] = build_nc()
    nc = _COMPILED['nc']
    in_maps = []
    for c in range(N_CORES):
        m = {'xs': np.ascontiguousarray(x[c * BPC:(c + 1) * BPC])}
        m.update(consts)
        in_maps.append(m)
    res = run_bass_kernel_spmd(
        nc, in_maps, core_ids=list(range(N_CORES)), trace=trace, **run_kwargs
    )
    out = np.concatenate([res.results[c]['y'] for c in range(N_CORES)], axis=0)
    return out, res


def kernel(**inputs):
    out, _ = run_on_cores(inputs)
    return out
